# revision 16
# baseline (speedup 1.0000x reference)
"""CapsNet forward pass as a fused Bass/Tile kernel on 8 Trainium2 NeuronCores.

Math (validated vs the jax reference to ~3e-7 rel):
  The dynamic-routing logits are never updated, so routing coefficients are
  uniform and the capsule stage collapses to a mean over the 1152 capsules:
      m[b,e]   = (1/1152) * sum_n (u[b,n,:] @ dig_W[n,:,e]) + sum_n dig_Wb[n,e]/1152
      scale[b] = l2/(1+l2)/l1   with l2=||m||_2, l1=||m||_1
      logits   = scale * (m @ sum_i out_w[:,i,:].T) + out_b ; softmax.
  Only conv1 (9x9, 1->256) and the primary-caps conv (9x9 s2, 32->32 per
  depth-slice d) need real compute.

Per-core structure (data parallel, 64 samples/core, 4 chunks of 16):
  im2col for conv1 is built HOST-side: IM[(p,q)->128, chunk, (y,x,b)] bf16,
  so the device only does 4 fully-contiguous HBM->SBUF loads (no gather).
  conv1 matmuls (lhsT=W1r[128,256], one output row y per matmul) evict with
  relu+bias straight into C1t[(d%4,ci), y//4, y%4, x, b] so the phase-shuffle
  DMAs (-> C1ph[(y%4,ci), d, y//4, x, b]) are contiguous 608B-run copies.
  prim conv: 33 K-tiles (11 window cols x 3 row segments) x 2x2 output
  blocking; the 99 prim matmuls of chunk k-1 are paced INSIDE chunk k's
  conv1 y-loop so the PE never idles on conv1's PSUM evictions.
  dig projection: 72 K-tiles -> m[16,64]; squash scalars; logits; softmax.
"""
import sys

sys.path.insert(0, "/opt/trn_rl_repo")

import numpy as np
import ml_dtypes

N_CORES = 8
B = 512
BC = B // N_CORES        # 64 samples per core
BCH = 16                 # batch chunk
NCH = BC // BCH          # 4 chunks


# ---------------------------------------------------------------- host tables
def _build_tables(conv1_w, conv1_b, prim_w, prim_b, dig_W, dig_Wb, out_w, out_b):
    w1 = conv1_w[:, 0].reshape(256, 81)
    w2 = prim_w[:, :, 0]

    # K padded 81->128 so the compiler's fast-weight-load path (needs full
    # 128-partition weights) kicks in for the conv1 matmuls.
    W1r = np.zeros((128, 256), np.float32)
    Cbias = np.zeros((128, 2), np.float32)
    for d in range(8):
        for ci in range(32):
            c = ci * 8 + d
            T, mu1 = d // 4, 32 * (d % 4) + ci
            W1r[:81, T * 128 + mu1] = w1[c]
            Cbias[mu1, T] = conv1_b[c]

    W2stk = np.zeros((33, 128, 128), np.float32)
    for xh in range(11):
        for seg in range(3):
            t = xh * 3 + seg
            nphi = 4 if seg < 2 else 3
            for phi in range(nphi):
                for s in range(4):
                    sy, sx = s // 2, s % 2
                    p = 4 * seg + phi - 2 * sy
                    q = xh - 2 * sx
                    if 0 <= p <= 8 and 0 <= q <= 8:
                        for ci in range(32):
                            W2stk[t, 32 * phi + ci, 32 * s:32 * s + 32] = w2[:, ci, p, q]

    Pbias = np.zeros((128, 1), np.float32)
    for s in range(4):
        Pbias[32 * s:32 * s + 32, 0] = prim_b

    Wdig = np.zeros((72, 128, 16), np.float32)
    t = 0
    for oy in range(3):
        for d in range(8):
            for ox in range(3):
                for s in range(4):
                    sy, sx = s // 2, s % 2
                    ip, jp = 2 * oy + sy, 2 * ox + sx
                    for co in range(32):
                        n = co * 36 + jp * 6 + ip
                        Wdig[t, 32 * s + co] = dig_W[n, d] / 1152.0
                t += 1

    bf = ml_dtypes.bfloat16
    return dict(
        W1r=W1r.astype(bf),
        Cbias=Cbias,
        W2stk=np.ascontiguousarray(W2stk.transpose(1, 0, 2)).astype(bf),  # [128,33,128]
        Pbias=Pbias,
        Wdig=np.ascontiguousarray(Wdig.transpose(1, 0, 2)).astype(bf),    # [128,72,16]
        Dbias=(dig_Wb.sum(0) / 1152.0).reshape(16, 1).astype(np.float32),
        W2sT=np.ascontiguousarray(out_w[..., 0].sum(1).T).astype(np.float32),  # [16,10]
        ob=np.tile(out_b[None, :], (BC, 1)).astype(np.float32),                # [64,10]
    )


def _build_im2col(x):
    """x: [B, 784] f32 -> per-core im2col [NCH, 128, 19*19*BCH] bf16.

    IM[core][k, (p,q), (y,x,b)] = x[core*64 + k*16 + b, (y+p)*28 + (x+q)],
    rows 81..127 zero (padding so the full-128-partition weight path is used).
    """
    xi = x.reshape(B, 28, 28)
    # windows[n, p, q, y, x] = xi[n, y+p, x+q] for y,x in 0..19
    w = np.lib.stride_tricks.sliding_window_view(xi, (9, 9), axis=(1, 2))
    # w: [B, 20, 20, 9, 9] -> take 19x19 outputs
    w = w[:, :19, :19]                                   # [B, 19, 19, 9, 9]
    cores = []
    for c in range(N_CORES):
        wc = w[c * BC:(c + 1) * BC]                      # [64, 19, 19, 9, 9]
        wc = wc.reshape(NCH, BCH, 19, 19, 81)
        # -> [NCH, (p,q)=81, y, x, b]
        wc = wc.transpose(0, 4, 2, 3, 1)
        out = np.zeros((NCH, 128, 19 * 19 * BCH), ml_dtypes.bfloat16)
        out[:, :81] = wc.reshape(NCH, 81, -1).astype(ml_dtypes.bfloat16)
        cores.append(out)
    return cores


# ---------------------------------------------------------------- bass kernel
def _build_nc():
    import concourse.bacc as bacc
    import concourse.bass as bass
    import concourse.mybir as mybir
    import concourse.tile as tile
    from concourse.masks import make_identity

    bf = mybir.dt.bfloat16
    f32 = mybir.dt.float32
    AF = mybir.ActivationFunctionType
    AX = mybir.AxisListType

    nc = bacc.Bacc(None, target_bir_lowering=False)

    IM_in = nc.dram_tensor("IM", [NCH, 128, 19 * 19 * BCH], bf, kind="ExternalInput")
    W1r_d = nc.dram_tensor("W1r", [128, 256], bf, kind="ExternalInput")
    W2stk_d = nc.dram_tensor("W2stk", [128, 33, 128], bf, kind="ExternalInput")
    Wdig_d = nc.dram_tensor("Wdig", [128, 72, 16], bf, kind="ExternalInput")
    Cbias_d = nc.dram_tensor("Cbias", [128, 2], f32, kind="ExternalInput")
    Pbias_d = nc.dram_tensor("Pbias", [128, 1], f32, kind="ExternalInput")
    Dbias_d = nc.dram_tensor("Dbias", [16, 1], f32, kind="ExternalInput")
    W2sT_d = nc.dram_tensor("W2sT", [16, 10], f32, kind="ExternalInput")
    ob_d = nc.dram_tensor("ob", [BC, 10], f32, kind="ExternalInput")
    out_d = nc.dram_tensor("out", [BC, 10], f32, kind="ExternalOutput")

    # last conv1 row needed by shuffle batch phi: phi 3->y15, 0->y16, 1->y17, 2->y18
    PHI_AT_Y = {15: 3, 16: 0, 17: 1, 18: 2}

    with tile.TileContext(nc) as tc:
        with (
            tc.tile_pool(name="consts", bufs=1) as consts,
            tc.tile_pool(name="work", bufs=2) as work,
            tc.tile_pool(name="usb", bufs=1) as usbp,
            tc.tile_pool(name="fin", bufs=1) as fin,
            tc.tile_pool(name="ps1", bufs=4, space="PSUM") as ps1p,
            tc.tile_pool(name="pspr", bufs=2, space="PSUM") as psprp,
            tc.tile_pool(name="psm", bufs=1, space="PSUM") as psmp,
            tc.tile_pool(name="pssm", bufs=1, space="PSUM") as pssmp,
            nc.allow_non_contiguous_dma("phase-shuffle copies are strided"),
        ):
            # ---- load constants + full im2col (host-built, contiguous)
            W1sb = consts.tile([128, 256], bf)
            nc.sync.dma_start(out=W1sb, in_=W1r_d[:, :])
            Cb = consts.tile([128, 2], f32)
            nc.scalar.dma_start(out=Cb, in_=Cbias_d[:, :])
            # one im2col tile per chunk so chunk-0 matmuls only depend on
            # chunk-0's own DMAs. chunk 0 gets both HWDGE rings to itself
            # (full HBM bandwidth -> first conv1 matmul asap); chunks 1-3 +
            # weight tables stream behind it on the SWDGE ring in the order
            # they are needed.
            imts = [consts.tile([128, 19, 19, BCH], bf, name=f"im{k}")
                    for k in range(NCH)]
            imvs = [t.rearrange("p y x b -> p (y x b)") for t in imts]
            nc.sync.dma_start(out=imvs[0][0:64, :], in_=IM_in[0, 0:64, :])
            nc.scalar.dma_start(out=imvs[0][64:128, :], in_=IM_in[0, 64:128, :])
            nc.gpsimd.dma_start(out=imvs[1][:, :], in_=IM_in[1, :, :])
            W2sb = consts.tile([128, 33, 128], bf)
            Wdsb = consts.tile([128, 72, 16], bf)

            # the remaining big loads are deferred into the chunk loop so the
            # startup window's HBM bandwidth serves only chunk 0/1 + the
            # engines' instruction-stream fetch.
            def deferred_loads(k):
                if k == 0:
                    nc.gpsimd.dma_start(out=W2sb, in_=W2stk_d[:, :, :])
                    nc.gpsimd.dma_start(out=imvs[2][:, :], in_=IM_in[2, :, :])
                elif k == 1:
                    nc.gpsimd.dma_start(out=imvs[3][:, :], in_=IM_in[3, :, :])
                    nc.gpsimd.dma_start(out=Wdsb, in_=Wdig_d[:, :, :])
            Pb = consts.tile([128, 1], f32)
            nc.scalar.dma_start(out=Pb, in_=Pbias_d[:, :])
            Db = consts.tile([16, 1], f32)
            nc.gpsimd.dma_start(out=Db, in_=Dbias_d[:, :])
            W2s = consts.tile([16, 10], f32)
            nc.gpsimd.dma_start(out=W2s, in_=W2sT_d[:, :])
            obt = consts.tile([BC, 10], f32)
            nc.gpsimd.dma_start(out=obt, in_=ob_d[:, :])

            idf = consts.tile([16, 16], f32)
            make_identity(nc, idf)


            U_sb = usbp.tile([128, 3, 8, 3, BC], bf)

            # ---- prim conv, chunk k: 3 oy-groups x (33 K-tile matmuls + evict)
            # returned as a flat list of thunks so the caller can pace them
            # between conv1 matmuls/evictions.
            def prim_thunks(k, C1ph):
                thunks = []
                for oy in range(3):
                    pspr = psprp.tile([128, 8, 3, BCH], f32, tag="pspr", name="pspr")

                    def mm(t, pspr=pspr, oy=oy):
                        xh, seg = t // 3, t % 3
                        rows = 128 if seg < 2 else 96
                        nc.tensor.matmul(
                            pspr[:, :, :, :],
                            W2sb[0:rows, t, :],
                            C1ph[0:rows, :, oy + seg, xh:xh + 9:4, :],
                            start=(t == 0), stop=(t == 32),
                        )

                    def ev(pspr=pspr, oy=oy, k=k):
                        udst = U_sb[:, oy, :, :, k * BCH:(k + 1) * BCH]
                        if oy == 0:
                            nc.scalar.activation(out=udst, in_=pspr, func=AF.Relu,
                                                 bias=Pb[:, 0:1], scale=1.0)
                        else:
                            nc.vector.tensor_scalar(
                                out=udst, in0=pspr, scalar1=Pb[:, 0:1], scalar2=0.0,
                                op0=mybir.AluOpType.add, op1=mybir.AluOpType.max)

                    for t in range(33):
                        thunks.append(lambda t=t, mm=mm: mm(t))
                    thunks.append(ev)
                return thunks

            # ---- conv1 chunk k with prim thunks of chunk k-1 paced inside
            def emit_chunk(k, thunks):
                # C1t[(d%4,ci), y%4, y//4, x, b]: evictions write contiguous
                # [128,19,16] rows; each shuffle read is one fully-contiguous
                # per-partition run (up to 3040B), so descriptor generation
                # and DMA queue time stay cheap.
                C1t = [work.tile([128, 4, 5, 19, BCH], bf, tag=f"c1t{T}",
                                 name=f"c1t{T}") for T in range(2)]
                C1ph = work.tile([128, 8, 5, 19, BCH], bf, tag="c1ph", name="c1ph",
                                 bufs=3)
                ntot = len(thunks)
                done = [0]

                def pace(frac):
                    want = int(round(ntot * frac))
                    while done[0] < want:
                        thunks[done[0]]()
                        done[0] += 1

                shuf_ring = [nc.sync, nc.gpsimd]
                nshuf = [0]
                step = 0
                for T in range(2):
                    if T == 1:
                        deferred_loads(k)
                    for y in range(19):
                        ps = ps1p.tile([128, 19, BCH], f32, tag="ps1", name="ps1")
                        nc.tensor.matmul(ps, W1sb[:, 128 * T:128 * (T + 1)],
                                         imts[k][:, y, :, :])
                        dst = C1t[T][:, y % 4, y // 4, :, :]
                        if y % 2 == 0:
                            nc.scalar.activation(out=dst, in_=ps, func=AF.Relu,
                                                 bias=Cb[:, T:T + 1], scale=1.0)
                        else:
                            nc.vector.tensor_scalar(
                                out=dst, in0=ps, scalar1=Cb[:, T:T + 1],
                                scalar2=0.0, op0=mybir.AluOpType.add,
                                op1=mybir.AluOpType.max)
                        phi = PHI_AT_Y.get(y)
                        if phi is not None:
                            ny = 4 if phi == 3 else 5
                            for dd in range(4):
                                eng = shuf_ring[nshuf[0] % 2]
                                nshuf[0] += 1
                                eng.dma_start(
                                    out=C1ph[32 * phi:32 * phi + 32, 4 * T + dd,
                                             0:ny, :, :],
                                    in_=C1t[T][32 * dd:32 * dd + 32, phi, 0:ny,
                                               :, :],
                                )
                        step += 1
                        pace(step / 38.0)
                return C1ph

            pending = []  # (k, C1ph) with prim not yet emitted; skew = 1 chunk
            for k in range(NCH):
                old = pending.pop(0) if pending else None
                thunks = prim_thunks(*old) if old is not None else []
                C1ph = emit_chunk(k, thunks)
                pending.append((k, C1ph))

            # ---- last chunk's prim interleaved with the dig projection
            # (dig oy-group only needs U[:, oy] of all chunks -> ready right
            # after that oy's prim evict)
            psm = psmp.tile([16, BC], f32, tag="psm")
            tdig = [0]

            def emit_dig_oy(oy):
                for d in range(8):
                    for ox in range(3):
                        t = tdig[0]
                        nc.tensor.matmul(psm, Wdsb[:, t, :], U_sb[:, oy, d, ox, :],
                                         start=(t == 0), stop=(t == 71))
                        tdig[0] += 1

            (k_last, C1ph_last) = pending[0]
            last_thunks = prim_thunks(k_last, C1ph_last)
            dume = fin.tile([1, 1], f32)
            for oy in range(3):
                for th in last_thunks[34 * oy:34 * (oy + 1)]:
                    th()
                emit_dig_oy(oy)
                if oy == 0:
                    # warm the Exp activation table while the PE grinds the
                    # epilogue matmuls so the softmax tail doesn't pay the
                    # ~1.3us table swap.
                    nc.scalar.activation(out=dume, in_=Cb[0:1, 0:1], func=AF.Exp)
            m_sb = fin.tile([16, BC], f32)
            nc.vector.tensor_scalar_add(out=m_sb, in0=psm, scalar1=Db[:, 0:1])

            # ---- squash scalars (need m transposed to [b, e])
            psT = pssmp.tile([BC, 16], f32, tag="pssm")
            nc.tensor.transpose(psT, m_sb, idf)
            mT = fin.tile([BC, 16], f32)
            nc.vector.tensor_copy(out=mT, in_=psT)
            sq = fin.tile([BC, 16], f32)
            nc.vector.tensor_mul(sq, mT, mT)
            l2 = fin.tile([BC, 1], f32)
            nc.vector.reduce_sum(l2, sq, axis=AX.X)
            nc.scalar.activation(out=l2, in_=l2, func=AF.Sqrt)
            l1 = fin.tile([BC, 1], f32)
            nc.vector.tensor_reduce(l1, mT, axis=AX.X, op=mybir.AluOpType.add,
                                    apply_absolute_value=True)
            den = fin.tile([BC, 1], f32)
            nc.vector.tensor_scalar_add(out=den, in0=l2, scalar1=1.0)
            nc.vector.tensor_mul(den, den, l1)
            rden = fin.tile([BC, 1], f32)
            nc.vector.reciprocal(rden, den)
            scl = fin.tile([BC, 1], f32)
            nc.vector.tensor_mul(scl, l2, rden)

            # ---- logits = scale * (mT @ W2sT) + ob ; softmax
            pslg = pssmp.tile([BC, 10], f32, tag="pssm")
            nc.tensor.matmul(pslg, m_sb, W2s)          # [BC,10] = m_sb.T @ W2s
            lg = fin.tile([BC, 10], f32)
            nc.vector.tensor_scalar_mul(out=lg, in0=pslg, scalar1=scl[:, 0:1])
            nc.vector.tensor_add(lg, lg, obt)
            # logits are O(1) here (squash output is unit-scale, out_w ~0.1),
            # so the max-subtraction stabilizer is unnecessary for fp32 exp.
            ex = fin.tile([BC, 10], f32)
            nc.scalar.activation(out=ex, in_=lg, func=AF.Exp)
            sm = fin.tile([BC, 1], f32)
            nc.vector.reduce_sum(sm, ex, axis=AX.X)
            rsm = fin.tile([BC, 1], f32)
            nc.vector.reciprocal(rsm, sm)
            outt = fin.tile([BC, 10], f32)
            nc.vector.tensor_scalar_mul(out=outt, in0=ex, scalar1=rsm[:, 0:1])
            nc.sync.dma_start(out=out_d[:, :], in_=outt)

    nc.finalize()
    return nc


_CACHE = {}


def kernel(**inputs):
    from concourse.bass_utils import run_bass_kernel_spmd

    np_in = {k: np.asarray(v) for k, v in inputs.items()}
    tabs = _build_tables(
        np_in["conv1_w"], np_in["conv1_b"], np_in["prim_w"], np_in["prim_b"],
        np_in["dig_W"], np_in["dig_Wb"], np_in["out_w"], np_in["out_b"],
    )
    x = np_in["x"][:, 0].reshape(B, 784).astype(np.float32)
    IMs = _build_im2col(x)

    if "nc" not in _CACHE:
        _CACHE["nc"] = _build_nc()
    nc = _CACHE["nc"]

    shared = {
        "W1r": tabs["W1r"], "W2stk": tabs["W2stk"], "Wdig": tabs["Wdig"],
        "Cbias": tabs["Cbias"], "Pbias": tabs["Pbias"], "Dbias": tabs["Dbias"],
        "W2sT": tabs["W2sT"], "ob": tabs["ob"],
    }
    in_maps = [dict(shared, IM=IMs[c]) for c in range(N_CORES)]
    res = run_bass_kernel_spmd(nc, in_maps, core_ids=list(range(N_CORES)),
                               **_CACHE.get("run_kwargs", {}))
    _CACHE["last_result"] = res
    out = np.concatenate([res.results[c]["out"] for c in range(N_CORES)], axis=0)
    return out.astype(np.float32)


# revision 18
# speedup vs baseline: 1.0848x; 1.0848x over previous
"""CapsNet forward pass as a fused Bass/Tile kernel on 8 Trainium2 NeuronCores.

Math (validated vs the jax reference to ~3e-7 rel):
  The dynamic-routing logits are never updated, so routing coefficients are
  uniform and the capsule stage collapses to a mean over the 1152 capsules:
      m[b,e]   = (1/1152) * sum_n (u[b,n,:] @ dig_W[n,:,e]) + sum_n dig_Wb[n,e]/1152
      scale[b] = l2/(1+l2)/l1   with l2=||m||_2, l1=||m||_1
      logits   = scale * (m @ sum_i out_w[:,i,:].T) + out_b ; softmax.
  Only conv1 (9x9, 1->256) and the primary-caps conv (9x9 s2, 32->32 per
  depth-slice d) need real compute.

Per-core structure (data parallel, 64 samples/core, 4 chunks of 16):
  im2col for conv1 is built HOST-side: IM[(p,q)->128, chunk, (y,x,b)] bf16,
  so the device only does 4 fully-contiguous HBM->SBUF loads (no gather).
  conv1 matmuls (lhsT=W1r[128,256], one output row y per matmul) evict with
  relu+bias straight into C1t[(d%4,ci), y//4, y%4, x, b] so the phase-shuffle
  DMAs (-> C1ph[(y%4,ci), d, y//4, x, b]) are contiguous 608B-run copies.
  prim conv: 33 K-tiles (11 window cols x 3 row segments) x 2x2 output
  blocking; the 99 prim matmuls of chunk k-1 are paced INSIDE chunk k's
  conv1 y-loop so the PE never idles on conv1's PSUM evictions.
  dig projection: 72 K-tiles -> m[16,64]; squash scalars; logits; softmax.
"""
import sys

sys.path.insert(0, "/opt/trn_rl_repo")

import numpy as np
import ml_dtypes

N_CORES = 8
B = 512
BC = B // N_CORES        # 64 samples per core
BCH = 16                 # batch chunk
NCH = BC // BCH          # 4 chunks


# ---------------------------------------------------------------- host tables
def _build_tables(conv1_w, conv1_b, prim_w, prim_b, dig_W, dig_Wb, out_w, out_b):
    w1 = conv1_w[:, 0].reshape(256, 81)
    w2 = prim_w[:, :, 0]

    # K padded 81->128 so the compiler's fast-weight-load path (needs full
    # 128-partition weights) kicks in for the conv1 matmuls.
    W1r = np.zeros((128, 256), np.float32)
    Cbias = np.zeros((128, 2), np.float32)
    for d in range(8):
        for ci in range(32):
            c = ci * 8 + d
            T, mu1 = d // 4, 32 * (d % 4) + ci
            W1r[:81, T * 128 + mu1] = w1[c]
            Cbias[mu1, T] = conv1_b[c]

    W2stk = np.zeros((33, 128, 128), np.float32)
    for xh in range(11):
        for seg in range(3):
            t = xh * 3 + seg
            nphi = 4 if seg < 2 else 3
            for phi in range(nphi):
                for s in range(4):
                    sy, sx = s // 2, s % 2
                    p = 4 * seg + phi - 2 * sy
                    q = xh - 2 * sx
                    if 0 <= p <= 8 and 0 <= q <= 8:
                        for ci in range(32):
                            W2stk[t, 32 * phi + ci, 32 * s:32 * s + 32] = w2[:, ci, p, q]

    Pbias = np.zeros((128, 1), np.float32)
    for s in range(4):
        Pbias[32 * s:32 * s + 32, 0] = prim_b

    Wdig = np.zeros((72, 128, 16), np.float32)
    t = 0
    for oy in range(3):
        for d in range(8):
            for ox in range(3):
                for s in range(4):
                    sy, sx = s // 2, s % 2
                    ip, jp = 2 * oy + sy, 2 * ox + sx
                    for co in range(32):
                        n = co * 36 + jp * 6 + ip
                        Wdig[t, 32 * s + co] = dig_W[n, d] / 1152.0
                t += 1

    bf = ml_dtypes.bfloat16
    return dict(
        W1r=W1r.astype(bf),
        Cbias=Cbias,
        W2stk=np.ascontiguousarray(W2stk.transpose(1, 0, 2)).astype(bf),  # [128,33,128]
        Pbias=Pbias,
        Wdig=np.ascontiguousarray(Wdig.transpose(1, 0, 2)).astype(bf),    # [128,72,16]
        Dbias=(dig_Wb.sum(0) / 1152.0).reshape(16, 1).astype(np.float32),
        W2sT=np.ascontiguousarray(out_w[..., 0].sum(1).T).astype(np.float32),  # [16,10]
        ob=np.tile(out_b[None, :], (BC, 1)).astype(np.float32),                # [64,10]
    )


def _build_im2col(x):
    """x: [B, 784] f32 -> per-core im2col [NCH, 128, 19*19*BCH] bf16.

    IM[core][k, (p,q), (y,x,b)] = x[core*64 + k*16 + b, (y+p)*28 + (x+q)],
    rows 81..127 zero (padding so the full-128-partition weight path is used).
    """
    xi = x.reshape(B, 28, 28)
    # windows[n, p, q, y, x] = xi[n, y+p, x+q] for y,x in 0..19
    w = np.lib.stride_tricks.sliding_window_view(xi, (9, 9), axis=(1, 2))
    # w: [B, 20, 20, 9, 9] -> take 19x19 outputs
    w = w[:, :19, :19]                                   # [B, 19, 19, 9, 9]
    cores = []
    for c in range(N_CORES):
        wc = w[c * BC:(c + 1) * BC]                      # [64, 19, 19, 9, 9]
        wc = wc.reshape(NCH, BCH, 19, 19, 81)
        # -> [NCH, (p,q)=81, y, x, b]
        wc = wc.transpose(0, 4, 2, 3, 1)
        out = np.zeros((NCH, 128, 19 * 19 * BCH), ml_dtypes.bfloat16)
        out[:, :81] = wc.reshape(NCH, 81, -1).astype(ml_dtypes.bfloat16)
        cores.append(out)
    return cores


# ---------------------------------------------------------------- bass kernel
def _build_nc():
    import concourse.bacc as bacc
    import concourse.bass as bass
    import concourse.mybir as mybir
    import concourse.tile as tile
    from concourse.masks import make_identity

    bf = mybir.dt.bfloat16
    f32 = mybir.dt.float32
    AF = mybir.ActivationFunctionType
    AX = mybir.AxisListType

    nc = bacc.Bacc(None, target_bir_lowering=False)

    IM_in = nc.dram_tensor("IM", [NCH, 128, 19 * 19 * BCH], bf, kind="ExternalInput")
    W1r_d = nc.dram_tensor("W1r", [128, 256], bf, kind="ExternalInput")
    W2stk_d = nc.dram_tensor("W2stk", [128, 33, 128], bf, kind="ExternalInput")
    Wdig_d = nc.dram_tensor("Wdig", [128, 72, 16], bf, kind="ExternalInput")
    Cbias_d = nc.dram_tensor("Cbias", [128, 2], f32, kind="ExternalInput")
    Pbias_d = nc.dram_tensor("Pbias", [128, 1], f32, kind="ExternalInput")
    Dbias_d = nc.dram_tensor("Dbias", [16, 1], f32, kind="ExternalInput")
    W2sT_d = nc.dram_tensor("W2sT", [16, 10], f32, kind="ExternalInput")
    ob_d = nc.dram_tensor("ob", [BC, 10], f32, kind="ExternalInput")
    out_d = nc.dram_tensor("out", [BC, 10], f32, kind="ExternalOutput")

    # last conv1 row needed by shuffle batch phi: phi 3->y15, 0->y16, 1->y17, 2->y18
    PHI_AT_Y = {15: 3, 16: 0, 17: 1, 18: 2}

    with tile.TileContext(nc) as tc:
        with (
            tc.tile_pool(name="consts", bufs=1) as consts,
            tc.tile_pool(name="work", bufs=2) as work,
            tc.tile_pool(name="usb", bufs=1) as usbp,
            tc.tile_pool(name="fin", bufs=1) as fin,
            tc.tile_pool(name="ps1", bufs=4, space="PSUM") as ps1p,
            tc.tile_pool(name="pspr", bufs=2, space="PSUM") as psprp,
            tc.tile_pool(name="psm", bufs=1, space="PSUM") as psmp,
            tc.tile_pool(name="pssm", bufs=1, space="PSUM") as pssmp,
            nc.allow_non_contiguous_dma("phase-shuffle copies are strided"),
        ):
            # ---- load constants + full im2col (host-built, contiguous)
            W1sb = consts.tile([128, 256], bf)
            nc.sync.dma_start(out=W1sb, in_=W1r_d[:, :])
            Cb = consts.tile([128, 2], f32)
            nc.scalar.dma_start(out=Cb, in_=Cbias_d[:, :])
            # one im2col tile per chunk so chunk-0 matmuls only depend on
            # chunk-0's own DMAs. chunk 0 gets both HWDGE rings to itself
            # (full HBM bandwidth -> first conv1 matmul asap); chunks 1-3 +
            # weight tables stream behind it on the SWDGE ring in the order
            # they are needed.
            imts = [consts.tile([128, 19, 19, BCH], bf, name=f"im{k}")
                    for k in range(NCH)]
            imvs = [t.rearrange("p y x b -> p (y x b)") for t in imts]
            nc.sync.dma_start(out=imvs[0][0:64, :], in_=IM_in[0, 0:64, :])
            nc.scalar.dma_start(out=imvs[0][64:128, :], in_=IM_in[0, 64:128, :])
            nc.gpsimd.dma_start(out=imvs[1][:, :], in_=IM_in[1, :, :])
            W2sb = consts.tile([128, 33, 128], bf)
            nc.gpsimd.dma_start(out=W2sb, in_=W2stk_d[:, :, :])
            nc.gpsimd.dma_start(out=imvs[2][:, :], in_=IM_in[2, :, :])
            nc.gpsimd.dma_start(out=imvs[3][:, :], in_=IM_in[3, :, :])
            Wdsb = consts.tile([128, 72, 16], bf)
            nc.gpsimd.dma_start(out=Wdsb, in_=Wdig_d[:, :, :])

            def deferred_loads(k):
                pass
            Pb = consts.tile([128, 1], f32)
            nc.scalar.dma_start(out=Pb, in_=Pbias_d[:, :])
            Db = consts.tile([16, 1], f32)
            nc.gpsimd.dma_start(out=Db, in_=Dbias_d[:, :])
            W2s = consts.tile([16, 10], f32)
            nc.gpsimd.dma_start(out=W2s, in_=W2sT_d[:, :])
            obt = consts.tile([BC, 10], f32)
            nc.gpsimd.dma_start(out=obt, in_=ob_d[:, :])

            idf = consts.tile([16, 16], f32)
            make_identity(nc, idf)


            U_sb = usbp.tile([128, 3, 8, 3, BC], bf)

            # ---- prim conv, chunk k: 3 oy-groups x (33 K-tile matmuls + evict)
            # returned as a flat list of thunks so the caller can pace them
            # between conv1 matmuls/evictions.
            def prim_thunks(k, C1ph):
                thunks = []
                for oy in range(3):
                    pspr = psprp.tile([128, 8, 3, BCH], f32, tag="pspr", name="pspr")

                    def mm(t, pspr=pspr, oy=oy):
                        xh, seg = t // 3, t % 3
                        rows = 128 if seg < 2 else 96
                        nc.tensor.matmul(
                            pspr[:, :, :, :],
                            W2sb[0:rows, t, :],
                            C1ph[0:rows, :, oy + seg, xh:xh + 9:4, :],
                            start=(t == 0), stop=(t == 32),
                        )

                    def ev(pspr=pspr, oy=oy, k=k):
                        udst = U_sb[:, oy, :, :, k * BCH:(k + 1) * BCH]
                        if oy == 0:
                            nc.scalar.activation(out=udst, in_=pspr, func=AF.Relu,
                                                 bias=Pb[:, 0:1], scale=1.0)
                        else:
                            nc.vector.tensor_scalar(
                                out=udst, in0=pspr, scalar1=Pb[:, 0:1], scalar2=0.0,
                                op0=mybir.AluOpType.add, op1=mybir.AluOpType.max)

                    for t in range(33):
                        thunks.append(lambda t=t, mm=mm: mm(t))
                    thunks.append(ev)
                return thunks

            # ---- conv1 chunk k with prim thunks of chunk k-1 paced inside
            # conv1 runs as 12 matmuls of N<=512 per half-T (PSUM-bank-sized)
            # instead of 19 per-y matmuls: fewer instructions (IRAM fetch is
            # ~4GB/s) and fewer, larger PSUM evictions.
            NC1 = 12
            CUTS = [min(512 * c, 19 * 19 * BCH) for c in range(NC1 + 1)]
            # shuffle for phase phi fires once every C1 row y=phi (mod 4) is
            # evicted; with 512-col blocks: block c covers rows < 512(c+1)/304
            PHI_AT_C = {9: [3], 10: [0, 1], 11: [2]}

            def emit_chunk(k, thunks):
                C1t = [work.tile([128, 19, 19, BCH], bf, tag=f"c1t{T}",
                                 name=f"c1t{T}") for T in range(2)]
                c1f = [C1t[T].rearrange("p y x b -> p (y x b)") for T in range(2)]
                C1ph = work.tile([128, 8, 5, 19, BCH], bf, tag="c1ph", name="c1ph",
                                 bufs=3)
                ntot = len(thunks)
                done = [0]

                def pace(frac):
                    want = int(round(ntot * frac))
                    while done[0] < want:
                        thunks[done[0]]()
                        done[0] += 1

                imf = imts[k].rearrange("p y x b -> p (y x b)")
                shuf_ring = [nc.sync, nc.gpsimd]
                nshuf = [0]
                step = 0
                for T in range(2):
                    for c in range(NC1):
                        lo, hi = CUTS[c], CUTS[c + 1]
                        ps = ps1p.tile([128, 512], f32, tag="ps1", name="ps1")
                        nc.tensor.matmul(ps[:, 0:hi - lo],
                                         W1sb[:, 128 * T:128 * (T + 1)],
                                         imf[:, lo:hi])
                        dst = c1f[T][:, lo:hi]
                        if c % 2 == 0:
                            nc.scalar.activation(out=dst, in_=ps[:, 0:hi - lo],
                                                 func=AF.Relu,
                                                 bias=Cb[:, T:T + 1], scale=1.0)
                        else:
                            nc.vector.tensor_scalar(
                                out=dst, in0=ps[:, 0:hi - lo],
                                scalar1=Cb[:, T:T + 1],
                                scalar2=0.0, op0=mybir.AluOpType.add,
                                op1=mybir.AluOpType.max)
                        for phi in PHI_AT_C.get(c, ()):
                            ny = 4 if phi == 3 else 5
                            for dd in range(4):
                                eng = shuf_ring[nshuf[0] % 2]
                                nshuf[0] += 1
                                eng.dma_start(
                                    out=C1ph[32 * phi:32 * phi + 32, 4 * T + dd,
                                             0:ny, :, :],
                                    in_=C1t[T][32 * dd:32 * dd + 32, phi::4,
                                               :, :],
                                )
                        step += 1
                        pace(step / (2.0 * NC1))
                return C1ph

            pending = []  # (k, C1ph) with prim not yet emitted; skew = 1 chunk
            for k in range(NCH):
                old = pending.pop(0) if pending else None
                thunks = prim_thunks(*old) if old is not None else []
                C1ph = emit_chunk(k, thunks)
                pending.append((k, C1ph))

            # ---- last chunk's prim interleaved with the dig projection
            # (dig oy-group only needs U[:, oy] of all chunks -> ready right
            # after that oy's prim evict)
            psm = psmp.tile([16, BC], f32, tag="psm")
            tdig = [0]

            def emit_dig_oy(oy):
                for d in range(8):
                    for ox in range(3):
                        t = tdig[0]
                        nc.tensor.matmul(psm, Wdsb[:, t, :], U_sb[:, oy, d, ox, :],
                                         start=(t == 0), stop=(t == 71))
                        tdig[0] += 1

            (k_last, C1ph_last) = pending[0]
            last_thunks = prim_thunks(k_last, C1ph_last)
            dume = fin.tile([1, 1], f32)
            for oy in range(3):
                for th in last_thunks[34 * oy:34 * (oy + 1)]:
                    th()
                emit_dig_oy(oy)
                if oy == 0:
                    # warm the Exp activation table while the PE grinds the
                    # epilogue matmuls so the softmax tail doesn't pay the
                    # ~1.3us table swap.
                    nc.scalar.activation(out=dume, in_=Cb[0:1, 0:1], func=AF.Exp)
            m_sb = fin.tile([16, BC], f32)
            nc.vector.tensor_scalar_add(out=m_sb, in0=psm, scalar1=Db[:, 0:1])

            # ---- squash scalars (need m transposed to [b, e])
            psT = pssmp.tile([BC, 16], f32, tag="pssm")
            nc.tensor.transpose(psT, m_sb, idf)
            mT = fin.tile([BC, 16], f32)
            nc.vector.tensor_copy(out=mT, in_=psT)
            sq = fin.tile([BC, 16], f32)
            nc.vector.tensor_mul(sq, mT, mT)
            l2 = fin.tile([BC, 1], f32)
            nc.vector.reduce_sum(l2, sq, axis=AX.X)
            nc.scalar.activation(out=l2, in_=l2, func=AF.Sqrt)
            l1 = fin.tile([BC, 1], f32)
            nc.vector.tensor_reduce(l1, mT, axis=AX.X, op=mybir.AluOpType.add,
                                    apply_absolute_value=True)
            den = fin.tile([BC, 1], f32)
            nc.vector.tensor_scalar_add(out=den, in0=l2, scalar1=1.0)
            nc.vector.tensor_mul(den, den, l1)
            rden = fin.tile([BC, 1], f32)
            nc.vector.reciprocal(rden, den)
            scl = fin.tile([BC, 1], f32)
            nc.vector.tensor_mul(scl, l2, rden)

            # ---- logits = scale * (mT @ W2sT) + ob ; softmax
            pslg = pssmp.tile([BC, 10], f32, tag="pssm")
            nc.tensor.matmul(pslg, m_sb, W2s)          # [BC,10] = m_sb.T @ W2s
            lg = fin.tile([BC, 10], f32)
            nc.vector.tensor_scalar_mul(out=lg, in0=pslg, scalar1=scl[:, 0:1])
            nc.vector.tensor_add(lg, lg, obt)
            # logits are O(1) here (squash output is unit-scale, out_w ~0.1),
            # so the max-subtraction stabilizer is unnecessary for fp32 exp.
            ex = fin.tile([BC, 10], f32)
            nc.scalar.activation(out=ex, in_=lg, func=AF.Exp)
            sm = fin.tile([BC, 1], f32)
            nc.vector.reduce_sum(sm, ex, axis=AX.X)
            rsm = fin.tile([BC, 1], f32)
            nc.vector.reciprocal(rsm, sm)
            outt = fin.tile([BC, 10], f32)
            nc.vector.tensor_scalar_mul(out=outt, in0=ex, scalar1=rsm[:, 0:1])
            nc.sync.dma_start(out=out_d[:, :], in_=outt)

    nc.finalize()
    return nc


_CACHE = {}


def kernel(**inputs):
    from concourse.bass_utils import run_bass_kernel_spmd

    np_in = {k: np.asarray(v) for k, v in inputs.items()}
    tabs = _build_tables(
        np_in["conv1_w"], np_in["conv1_b"], np_in["prim_w"], np_in["prim_b"],
        np_in["dig_W"], np_in["dig_Wb"], np_in["out_w"], np_in["out_b"],
    )
    x = np_in["x"][:, 0].reshape(B, 784).astype(np.float32)
    IMs = _build_im2col(x)

    if "nc" not in _CACHE:
        _CACHE["nc"] = _build_nc()
    nc = _CACHE["nc"]

    shared = {
        "W1r": tabs["W1r"], "W2stk": tabs["W2stk"], "Wdig": tabs["Wdig"],
        "Cbias": tabs["Cbias"], "Pbias": tabs["Pbias"], "Dbias": tabs["Dbias"],
        "W2sT": tabs["W2sT"], "ob": tabs["ob"],
    }
    in_maps = [dict(shared, IM=IMs[c]) for c in range(N_CORES)]
    res = run_bass_kernel_spmd(nc, in_maps, core_ids=list(range(N_CORES)),
                               **_CACHE.get("run_kwargs", {}))
    _CACHE["last_result"] = res
    out = np.concatenate([res.results[c]["out"] for c in range(N_CORES)], axis=0)
    return out.astype(np.float32)
